# revision 14
# baseline (speedup 1.0000x reference)
"""Multi-head attention kernel for 8 Trainium2 NeuronCores.

Problem: B=2, S=2048, D=1024, H=16 heads (Dh=64).
    qh = split(q @ wq.T + bq); kh, vh likewise
    out = concat_h(softmax(qh kh^T / 8) vh) @ wo.T + bo

Sharding: core c = 4*b + j handles batch b and head group j (4 heads,
channels [256j, 256j+256)).  Each core computes its 4 heads' attention and
a partial output projection; the host sums the 4 partials per batch and
adds the constant bv @ wo.T + bo vector.

v2: the ACT exp stream (16.8M exps/core at 1 elem/cycle/lane @1.2GHz ~
144us) is the bottleneck engine, so everything is scheduled around it:
  - Only K-proj(tb0,cc0) + Q-proj(qb0,cc0) gate the first scores/exp;
    all other projection work is deferred into the attention chunks'
    kt loops (one unit per kt slot, deadline-ordered), where the Tile
    greedy scheduler slots it into PE gaps while ACT streams exp.
  - DMAs on two queues (sync: critical lead-in + output; gpsimd: rest
    in need-order), so the exp stream starts ~8us in.
  - PV is col-tiled: the head pair runs concurrently (M=64+64 on col
    strips 0-1/2-3), softmax denominators are separate M=32 ones-lhsT
    matmuls whose PSUM homes alternate by kt parity so consecutive-kt
    den quads pack into one PE window: 3 PE windows per 2 kt.
  - exp on ACT straight out of PSUM (no max subtraction: scores ~N(0,1)
    after the 1/8 scale), bf16 matmuls with fp32 PSUM accumulation.
"""

import numpy as np
import ml_dtypes
import concourse.bass as bass
import concourse.tile as tile
import concourse.mybir as mybir
from concourse import bacc, bass_utils

B, S, D, H = 2, 2048, 1024, 16
DH = 64
HL = 4            # heads per core
CL = HL * DH      # local channels = 256
N_CORES = 8

f32 = mybir.dt.float32
bf16 = mybir.dt.bfloat16
AF = mybir.ActivationFunctionType
BF = ml_dtypes.bfloat16

TB = 4            # token blocks for projections (512 tokens each)
TBS = S // TB     # 512
QB = 4            # query blocks for attention (512 queries each)
QBS = S // QB     # 512
KT_N = S // 128   # 16 key tiles


def build():
    nc = bacc.Bacc("TRN2", debug=False, num_devices=N_CORES)
    qT = nc.dram_tensor("qT", [D, S], bf16, kind="ExternalInput").ap()
    kT = nc.dram_tensor("kT", [D, S], bf16, kind="ExternalInput").ap()
    vT = nc.dram_tensor("vT", [D, S], bf16, kind="ExternalInput").ap()
    wqT = nc.dram_tensor("wqT", [D, CL], bf16, kind="ExternalInput").ap()
    wkT = nc.dram_tensor("wkT", [D, CL], bf16, kind="ExternalInput").ap()
    wvT = nc.dram_tensor("wvT", [D, CL], bf16, kind="ExternalInput").ap()
    woT = nc.dram_tensor("woT", [CL, D], bf16, kind="ExternalInput").ap()
    bq = nc.dram_tensor("bq", [CL], f32, kind="ExternalInput").ap()
    bk = nc.dram_tensor("bk", [CL], f32, kind="ExternalInput").ap()
    out = nc.dram_tensor("out", [S, D], f32, kind="ExternalOutput").ap()

    with tile.TileContext(nc) as tc:
        with (
            tc.tile_pool(name="wp", bufs=1) as wp,
            tc.tile_pool(name="xp", bufs=8) as xp,
            tc.tile_pool(name="qkv", bufs=1) as qkv,
            tc.tile_pool(name="cp", bufs=1) as cp,
            tc.tile_pool(name="ep", bufs=10) as ep,
            tc.tile_pool(name="rp", bufs=2) as rp,
            tc.tile_pool(name="op", bufs=2) as op,
            tc.tile_pool(name="pp", bufs=2, space="PSUM") as pp,
            tc.tile_pool(name="sp", bufs=2, space="PSUM") as sp,
            tc.tile_pool(name="cps", bufs=1, space="PSUM") as cps,
        ):
            # ---- ACT exp-table preload (hides the ~2.7us table load) ----
            ones_sb = wp.tile([128, 32], bf16)
            nc.vector.memset(ones_sb, 1.0)
            scratch = wp.tile([128, 32], bf16)
            nc.scalar.activation(scratch, ones_sb, AF.Exp)

            # ---- weights/biases; input DMAs in need-order across 2 queues ----
            wk_sb = wp.tile([128, 8, CL], bf16)
            wq_sb = wp.tile([128, 8, CL], bf16)
            wv_sb = wp.tile([128, 8, CL], bf16)
            wo_sb = wp.tile([128, 2, D], bf16)
            bq_sb = wp.tile([128, 2], f32)
            bk_sb = wp.tile([128, 2], f32)
            nc.sync.dma_start(wk_sb, wkT.rearrange("(c p) n -> p c n", p=128))
            nc.gpsimd.dma_start(wq_sb, wqT.rearrange("(c p) n -> p c n", p=128))
            nc.gpsimd.dma_start(bk_sb, bk.rearrange("(c p) -> p c", p=128))
            nc.gpsimd.dma_start(bq_sb, bq.rearrange("(c p) -> p c", p=128))

            def x_load(src, tb, eng, name, gate=None):
                t0 = tb * TBS
                x = xp.tile([128, 8, TBS], bf16, tag="x", name=name)
                if gate is not None:
                    # tiny write into the tile, dep on the last critical DMA:
                    # holds this transfer (and, FIFO, the rest of its queue)
                    # until the critical lead-in stream has full bandwidth
                    nc.gpsimd.tensor_copy(x[0:1, 0, 0:1], gate)
                eng.dma_start(x, src.rearrange("(c p) t -> p c t", p=128)[:, :, t0:t0 + TBS])
                return x

            xk = [None] * TB
            xq = [None] * TB
            xk[0] = x_load(kT, 0, nc.sync, "xk")
            xq[0] = x_load(qT, 0, nc.gpsimd, "xq")
            gate = xq[0][0:1, 0, 0:1]
            xk[1] = x_load(kT, 1, nc.gpsimd, "xk", gate)
            xk[2] = x_load(kT, 2, nc.gpsimd, "xk", gate)
            nc.gpsimd.tensor_copy(wv_sb[0:1, 0, 0:1], gate)
            nc.gpsimd.dma_start(wv_sb, wvT.rearrange("(c p) n -> p c n", p=128))
            xv = [None] * TB
            xv[0] = x_load(vT, 0, nc.gpsimd, "xv", gate)
            xk[3] = x_load(kT, 3, nc.gpsimd, "xk", gate)
            xv[1] = x_load(vT, 1, nc.gpsimd, "xv", gate)
            xv[2] = x_load(vT, 2, nc.gpsimd, "xv", gate)
            xv[3] = x_load(vT, 3, nc.gpsimd, "xv", gate)
            nc.gpsimd.tensor_copy(wo_sb[0:1, 0, 0:1], gate)
            nc.gpsimd.dma_start(wo_sb, woT.rearrange("(c p) n -> p c n", p=128))

            QT = qkv.tile([128, 2, S], bf16)   # [chan, tok]
            KT = qkv.tile([128, 2, S], bf16)
            V = qkv.tile([128, KT_N, CL], bf16)  # [tok128, kt, chan]

            def make_kq_cc(w_sb, b_sb, x, dst, tb, cc):
                def unit():
                    t0 = tb * TBS
                    ps = pp.tile([128, TBS], f32, tag="pp", name="ps_kq")
                    for c in range(8):
                        nc.tensor.matmul(ps, w_sb[:, c, cc * 128:(cc + 1) * 128],
                                         x[:, c], start=(c == 0), stop=(c == 7))
                    nc.vector.tensor_scalar_add(dst[:, cc, t0:t0 + TBS], ps,
                                                b_sb[:, cc:cc + 1])
                return unit

            def make_v_tt(tb, tt):
                def unit():
                    ps = pp.tile([128, CL], f32, tag="pp", name="ps_v")
                    for c in range(8):
                        nc.tensor.matmul(ps, xv[tb][:, c, tt * 128:(tt + 1) * 128],
                                         wv_sb[:, c], start=(c == 0), stop=(c == 7))
                    nc.vector.tensor_copy(V[:, tb * 4 + tt, :], ps)
                return unit

            # lead-in: only these two gate the first exp
            make_kq_cc(wk_sb, bk_sb, xk[0], KT, 0, 0)()
            make_kq_cc(wq_sb, bq_sb, xq[0], QT, 0, 0)()

            vu = [make_v_tt(tb, tt) for tb in range(TB) for tt in range(4)]
            deferred = [
                make_kq_cc(wk_sb, bk_sb, xk[1], KT, 1, 0), vu[0], vu[1],
                make_kq_cc(wk_sb, bk_sb, xk[2], KT, 2, 0), vu[2], vu[3], vu[4],
                make_kq_cc(wk_sb, bk_sb, xk[3], KT, 3, 0), vu[5], vu[6], vu[7],
                vu[8],
                make_kq_cc(wq_sb, bq_sb, xq[0], QT, 0, 1),
                make_kq_cc(wk_sb, bk_sb, xk[0], KT, 0, 1), vu[9],
                make_kq_cc(wk_sb, bk_sb, xk[1], KT, 1, 1), vu[10], vu[11],
                make_kq_cc(wk_sb, bk_sb, xk[2], KT, 2, 1), vu[12], vu[13],
                make_kq_cc(wk_sb, bk_sb, xk[3], KT, 3, 1), vu[14], vu[15],
            ]

            # ---- attention + output projection ----
            C = cp.tile([128, 2, S], bf16)   # C^T [cat-chan, tok]

            def make_norm(hp, q0, c_ab, den):
                def norm():
                    # den homes: a_even p0:32 | b_even p32:64 | a_odd p64:96 | b_odd p96:128
                    t64 = rp.tile([64, QBS], f32, tag="t64", name="t64")
                    nc.vector.tensor_copy(t64, den[64:128, :])
                    da = rp.tile([64, QBS], f32, tag="da", name="da")
                    nc.vector.tensor_add(da, den[0:64, :], t64)
                    r = rp.tile([128, QBS], f32, tag="r", name="r")
                    nc.vector.reciprocal_approx_fast(r[0:64, :], da)
                    # replicate to a|a|b|b matching c_ab channel rows
                    nc.vector.tensor_copy(r[96:128, :], r[32:64, :])
                    nc.vector.tensor_copy(r[32:64, :], r[0:32, :])
                    nc.vector.tensor_copy(r[64:96, :], r[96:128, :])
                    nc.vector.tensor_mul(C[:, hp, q0:q0 + QBS], c_ab, r)
                return norm

            def make_outproj_tt(q0, tt):
                def outproj():
                    tg = q0 + tt * 128
                    o = op.tile([128, D], f32, tag="o")
                    ps0 = pp.tile([128, 512], f32, tag="pp", name="ps0")
                    ps1 = pp.tile([128, 512], f32, tag="pp", name="ps1")
                    for cc in range(2):
                        nc.tensor.matmul(ps0, C[:, cc, tg:tg + 128],
                                         wo_sb[:, cc, 0:512],
                                         start=(cc == 0), stop=(cc == 1))
                        nc.tensor.matmul(ps1, C[:, cc, tg:tg + 128],
                                         wo_sb[:, cc, 512:1024],
                                         start=(cc == 0), stop=(cc == 1))
                    nc.vector.tensor_copy(o[:, 0:512], ps0)
                    nc.vector.tensor_copy(o[:, 512:1024], ps1)
                    nc.sync.dma_start(out[tg:tg + 128, :], o)
                return outproj

            def make_qproj_late(tb, cc):
                def unit():
                    if cc == 0:
                        xq[tb] = x_load(qT, tb, nc.gpsimd, "xq")
                    make_kq_cc(wq_sb, bq_sb, xq[tb], QT, tb, cc)()
                return unit

            for qb in range(QB):
                q0 = qb * QBS
                for hp in range(2):
                    c_ab = cps.tile([128, QBS], f32, tag="cab", name="c_ab")
                    den = cps.tile([128, QBS], f32, tag="den", name="den")

                    def pv2(k0, e0, k1, e1, c_ab=c_ab, den=den, hp=hp):
                        # one atomic 2-kt group; strip-balanced order packs
                        # into 3 PE windows: [PVa|PVb](k0), [PVa|PVb](k1),
                        # [dena_e|denb_e|dena_o|denb_o]
                        h0 = 128 * hp
                        nc.tensor.matmul(c_ab[0:64, :], V[:, k0, h0:h0 + 64],
                                         e0[:, 0:QBS], start=(k0 == 0),
                                         stop=(k0 == KT_N - 1))
                        # tail of the group at priority 0: keeps the 8 MMs
                        # adjacent in the static PE order so col-strip
                        # concurrency packs them into 3 PE windows
                        with tc.high_priority():
                            nc.tensor.matmul(c_ab[64:128, :],
                                             V[:, k0, h0 + 64:h0 + 128],
                                             e0[:, QBS:2 * QBS], start=(k0 == 0),
                                             stop=(k0 == KT_N - 1))
                            nc.tensor.matmul(c_ab[0:64, :],
                                             V[:, k1, h0:h0 + 64],
                                             e1[:, 0:QBS], start=(k1 == 0),
                                             stop=(k1 == KT_N - 1))
                            nc.tensor.matmul(c_ab[64:128, :],
                                             V[:, k1, h0 + 64:h0 + 128],
                                             e1[:, QBS:2 * QBS], start=(k1 == 0),
                                             stop=(k1 == KT_N - 1))
                            for kt, e in ((k0, e0), (k1, e1)):
                                p = 64 * (kt % 2)   # parity-alternating homes
                                nc.tensor.matmul(den[p:p + 32, :], ones_sb,
                                                 e[:, 0:QBS], start=(kt < 2),
                                                 stop=(kt >= KT_N - 2),
                                                 tile_position=(0, p))
                                nc.tensor.matmul(den[p + 32:p + 64, :], ones_sb,
                                                 e[:, QBS:2 * QBS], start=(kt < 2),
                                                 stop=(kt >= KT_N - 2),
                                                 tile_position=(0, p + 32))

                    # first chunk drains 2 units/kt so every V-proj unit is
                    # emitted before the PV flush that consumes it
                    drain = 2 if (qb == 0 and hp == 0) else 1
                    pending = []
                    for kt in range(KT_N):
                        k0 = kt * 128
                        s_ps = sp.tile([128, 2 * QBS], f32, tag="s")
                        nc.tensor.matmul(s_ps[:, 0:QBS],
                                         KT[0:64, hp, k0:k0 + 128],
                                         QT[0:64, hp, q0:q0 + QBS])
                        with tc.high_priority():
                            nc.tensor.matmul(s_ps[:, QBS:2 * QBS],
                                             KT[64:128, hp, k0:k0 + 128],
                                             QT[64:128, hp, q0:q0 + QBS])
                        e = ep.tile([128, 2 * QBS], bf16, tag="e")
                        nc.scalar.activation(e, s_ps, AF.Exp, scale=0.125)
                        pending.append((kt, e))
                        if kt >= 1:
                            for _ in range(drain):
                                if deferred:
                                    deferred.pop(0)()
                        if len(pending) >= 7:
                            a = pending.pop(0)
                            b = pending.pop(0)
                            pv2(a[0], a[1], b[0], b[1])
                    while pending:
                        a = pending.pop(0)
                        b = pending.pop(0)
                        pv2(a[0], a[1], b[0], b[1])
                    deferred.insert(0, make_norm(hp, q0, c_ab, den))
                    if qb + 1 < QB:
                        deferred.insert(1, make_qproj_late(qb + 1, hp))
                for tt in range(4):
                    deferred.append(make_outproj_tt(q0, tt))
            for fn in deferred:
                fn()

    nc.compile()
    return nc


_CACHE = {}


def _get_nc():
    if "nc" not in _CACHE:
        _CACHE["nc"] = build()
    return _CACHE["nc"]


def make_in_maps(q, k, v, wq, bq, wk, bk, wv, bv, wo, bo):
    xT = {}
    for b in range(B):
        xT[("q", b)] = np.ascontiguousarray(np.asarray(q[b]).T).astype(BF)
        xT[("k", b)] = np.ascontiguousarray(np.asarray(k[b]).T).astype(BF)
        xT[("v", b)] = np.ascontiguousarray(np.asarray(v[b]).T).astype(BF)
    in_maps = []
    for core in range(N_CORES):
        b, j = divmod(core, N_CORES // B)
        sl = slice(CL * j, CL * (j + 1))
        in_maps.append({
            "qT": xT[("q", b)],
            "kT": xT[("k", b)],
            "vT": xT[("v", b)],
            "wqT": np.ascontiguousarray(np.asarray(wq)[sl, :].T).astype(BF),
            "wkT": np.ascontiguousarray(np.asarray(wk)[sl, :].T).astype(BF),
            "wvT": np.ascontiguousarray(np.asarray(wv)[sl, :].T).astype(BF),
            "woT": np.ascontiguousarray(np.asarray(wo)[:, sl].T).astype(BF),
            "bq": np.ascontiguousarray(bq[sl], dtype=np.float32),
            "bk": np.ascontiguousarray(bk[sl], dtype=np.float32),
        })
    return in_maps


def combine(results, bv, wo, bo):
    GP = N_CORES // B
    const = (np.asarray(bv, dtype=np.float64) @ np.asarray(wo, dtype=np.float64).T
             + np.asarray(bo, dtype=np.float64)).astype(np.float32)
    out = np.empty((B, S, D), dtype=np.float32)
    for b in range(B):
        acc = results[b * GP]["out"].astype(np.float32).copy()
        for j in range(1, GP):
            acc += results[b * GP + j]["out"]
        out[b] = acc + const[None, :]
    return out


def kernel(q, k, v, wq, bq, wk, bk, wv, bv, wo, bo):
    nc = _get_nc()
    in_maps = make_in_maps(q, k, v, wq, bq, wk, bk, wv, bv, wo, bo)
    res = bass_utils.run_bass_kernel_spmd(nc, in_maps, core_ids=list(range(N_CORES)))
    return combine(res.results, bv, wo, bo)


# revision 18
# speedup vs baseline: 1.0840x; 1.0840x over previous
"""Multi-head attention kernel for 8 Trainium2 NeuronCores.

Problem: B=2, S=2048, D=1024, H=16 heads (Dh=64).
    qh = split(q @ wq.T + bq); kh, vh likewise
    out = concat_h(softmax(qh kh^T / 8) vh) @ wo.T + bo

Sharding: core c = 4*b + j handles batch b and head group j (4 heads,
channels [256j, 256j+256)).  Each core computes its 4 heads' attention and
a partial output projection; the host sums the 4 partials per batch and
adds the constant bv @ wo.T + bo vector.

v2: the ACT exp stream (16.8M exps/core at 1 elem/cycle/lane @1.2GHz ~
144us) is the bottleneck engine, so everything is scheduled around it:
  - Only K-proj(tb0,cc0) + Q-proj(qb0,cc0) gate the first scores/exp;
    all other projection work is deferred into the attention chunks'
    kt loops (one unit per kt slot, deadline-ordered), where the Tile
    greedy scheduler slots it into PE gaps while ACT streams exp.
  - DMAs on two queues (sync: critical lead-in + output; gpsimd: rest
    in need-order), so the exp stream starts ~8us in.
  - PV is col-tiled: the head pair runs concurrently (M=64+64 on col
    strips 0-1/2-3), softmax denominators are separate M=32 ones-lhsT
    matmuls whose PSUM homes alternate by kt parity so consecutive-kt
    den quads pack into one PE window: 3 PE windows per 2 kt.
  - exp on ACT straight out of PSUM (no max subtraction: scores ~N(0,1)
    after the 1/8 scale), bf16 matmuls with fp32 PSUM accumulation.
"""

import numpy as np
import ml_dtypes
import concourse.bass as bass
import concourse.tile as tile
import concourse.mybir as mybir
from concourse import bacc, bass_utils

B, S, D, H = 2, 2048, 1024, 16
DH = 64
HL = 4            # heads per core
CL = HL * DH      # local channels = 256
N_CORES = 8

f32 = mybir.dt.float32
bf16 = mybir.dt.bfloat16
AF = mybir.ActivationFunctionType
BF = ml_dtypes.bfloat16

TB = 4            # token blocks for projections (512 tokens each)
TBS = S // TB     # 512
QB = 4            # query blocks for attention (512 queries each)
QBS = S // QB     # 512
KT_N = S // 128   # 16 key tiles


def build():
    nc = bacc.Bacc("TRN2", debug=False, num_devices=N_CORES)
    qT = nc.dram_tensor("qT", [D, S], bf16, kind="ExternalInput").ap()
    kT = nc.dram_tensor("kT", [D, S], bf16, kind="ExternalInput").ap()
    vT = nc.dram_tensor("vT", [D, S], bf16, kind="ExternalInput").ap()
    wqT = nc.dram_tensor("wqT", [D, CL], bf16, kind="ExternalInput").ap()
    wkT = nc.dram_tensor("wkT", [D, CL], bf16, kind="ExternalInput").ap()
    wvT = nc.dram_tensor("wvT", [D, CL], bf16, kind="ExternalInput").ap()
    woT = nc.dram_tensor("woT", [CL, D], bf16, kind="ExternalInput").ap()
    bq = nc.dram_tensor("bq", [CL], f32, kind="ExternalInput").ap()
    bk = nc.dram_tensor("bk", [CL], f32, kind="ExternalInput").ap()
    out = nc.dram_tensor("out", [S, D], f32, kind="ExternalOutput").ap()

    with tile.TileContext(nc) as tc:
        with (
            tc.tile_pool(name="wp", bufs=1) as wp,
            tc.tile_pool(name="xp", bufs=8) as xp,
            tc.tile_pool(name="qkv", bufs=1) as qkv,
            tc.tile_pool(name="cp", bufs=1) as cp,
            tc.tile_pool(name="ep", bufs=10) as ep,
            tc.tile_pool(name="rp", bufs=2) as rp,
            tc.tile_pool(name="op", bufs=2) as op,
            tc.tile_pool(name="pp", bufs=2, space="PSUM") as pp,
            tc.tile_pool(name="sp", bufs=2, space="PSUM") as sp,
            tc.tile_pool(name="cps", bufs=1, space="PSUM") as cps,
        ):
            # ---- ACT exp-table preload (hides the ~2.7us table load) ----
            ones_sb = wp.tile([128, 32], bf16)
            nc.vector.memset(ones_sb, 1.0)
            scratch = wp.tile([128, 32], bf16)
            nc.scalar.activation(scratch, ones_sb, AF.Exp)

            # ---- weights/biases; input DMAs in need-order across 2 queues ----
            wk_sb = wp.tile([128, 8, CL], bf16)
            wq_sb = wp.tile([128, 8, CL], bf16)
            wv_sb = wp.tile([128, 8, CL], bf16)
            wo_sb = wp.tile([128, 2, D], bf16)
            bq_sb = wp.tile([128, 2], f32)
            bk_sb = wp.tile([128, 2], f32)
            nc.sync.dma_start(wk_sb, wkT.rearrange("(c p) n -> p c n", p=128))
            nc.gpsimd.dma_start(wq_sb, wqT.rearrange("(c p) n -> p c n", p=128))
            nc.gpsimd.dma_start(bk_sb, bk.rearrange("(c p) -> p c", p=128))
            nc.gpsimd.dma_start(bq_sb, bq.rearrange("(c p) -> p c", p=128))

            def x_load(src, tb, eng, name, gate=None):
                t0 = tb * TBS
                x = xp.tile([128, 8, TBS], bf16, tag="x", name=name)
                if gate is not None:
                    # tiny write into the tile, dep on the last critical DMA:
                    # holds this transfer (and, FIFO, the rest of its queue)
                    # until the critical lead-in stream has full bandwidth
                    nc.gpsimd.tensor_copy(x[0:1, 0, 0:1], gate)
                eng.dma_start(x, src.rearrange("(c p) t -> p c t", p=128)[:, :, t0:t0 + TBS])
                return x

            xk = [None] * TB
            xq = [None] * TB
            xk[0] = x_load(kT, 0, nc.sync, "xk")
            xq[0] = x_load(qT, 0, nc.gpsimd, "xq")
            gate = xq[0][0:1, 0, 0:1]
            xk[1] = x_load(kT, 1, nc.gpsimd, "xk", gate)
            xk[2] = x_load(kT, 2, nc.gpsimd, "xk", gate)
            nc.gpsimd.tensor_copy(wv_sb[0:1, 0, 0:1], gate)
            nc.gpsimd.dma_start(wv_sb, wvT.rearrange("(c p) n -> p c n", p=128))
            xv = [None] * TB
            xv[0] = x_load(vT, 0, nc.gpsimd, "xv", gate)
            xk[3] = x_load(kT, 3, nc.gpsimd, "xk", gate)
            xv[1] = x_load(vT, 1, nc.gpsimd, "xv", gate)
            xv[2] = x_load(vT, 2, nc.gpsimd, "xv", gate)
            xv[3] = x_load(vT, 3, nc.gpsimd, "xv", gate)
            nc.gpsimd.tensor_copy(wo_sb[0:1, 0, 0:1], gate)
            nc.gpsimd.dma_start(wo_sb, woT.rearrange("(c p) n -> p c n", p=128))

            QT = qkv.tile([128, 2, S], bf16)   # [chan, tok]
            KT = qkv.tile([128, 2, S], bf16)
            V = qkv.tile([128, KT_N, CL], bf16)  # [tok128, kt, chan]

            def make_kq_cc(w_sb, b_sb, x, dst, tb, cc):
                def unit():
                    t0 = tb * TBS
                    ps = pp.tile([128, TBS], f32, tag="pp", name="ps_kq")
                    for c in range(8):
                        nc.tensor.matmul(ps, w_sb[:, c, cc * 128:(cc + 1) * 128],
                                         x[:, c], start=(c == 0), stop=(c == 7))
                    nc.vector.tensor_scalar_add(dst[:, cc, t0:t0 + TBS], ps,
                                                b_sb[:, cc:cc + 1])
                return unit

            def make_v_tt(tb, tt):
                def unit():
                    ps = pp.tile([128, CL], f32, tag="pp", name="ps_v")
                    for c in range(8):
                        nc.tensor.matmul(ps, xv[tb][:, c, tt * 128:(tt + 1) * 128],
                                         wv_sb[:, c], start=(c == 0), stop=(c == 7))
                    nc.vector.tensor_copy(V[:, tb * 4 + tt, :], ps)
                return unit

            # lead-in: only these two gate the first exp
            make_kq_cc(wk_sb, bk_sb, xk[0], KT, 0, 0)()
            make_kq_cc(wq_sb, bq_sb, xq[0], QT, 0, 0)()

            vu = [make_v_tt(tb, tt) for tb in range(TB) for tt in range(4)]
            deferred = [
                make_kq_cc(wk_sb, bk_sb, xk[1], KT, 1, 0), vu[0], vu[1],
                make_kq_cc(wk_sb, bk_sb, xk[2], KT, 2, 0), vu[2], vu[3], vu[4],
                make_kq_cc(wk_sb, bk_sb, xk[3], KT, 3, 0), vu[5], vu[6], vu[7],
                vu[8],
                make_kq_cc(wq_sb, bq_sb, xq[0], QT, 0, 1),
                make_kq_cc(wk_sb, bk_sb, xk[0], KT, 0, 1), vu[9],
                make_kq_cc(wk_sb, bk_sb, xk[1], KT, 1, 1), vu[10], vu[11],
                make_kq_cc(wk_sb, bk_sb, xk[2], KT, 2, 1), vu[12], vu[13],
                make_kq_cc(wk_sb, bk_sb, xk[3], KT, 3, 1), vu[14], vu[15],
            ]

            # ---- attention + output projection ----
            C = cp.tile([128, 2, S], bf16)   # C^T [cat-chan, tok]

            def make_norm(hp, q0, c_ab, den):
                def norm():
                    # den: a at p0:32, b at p32:64
                    r = rp.tile([128, QBS], f32, tag="r", name="r")
                    nc.vector.reciprocal_approx_fast(r[0:64, :], den[0:64, :])
                    # replicate to a|a|b|b matching c_ab channel rows
                    nc.vector.tensor_copy(r[96:128, :], r[32:64, :])
                    nc.vector.tensor_copy(r[32:64, :], r[0:32, :])
                    nc.vector.tensor_copy(r[64:96, :], r[96:128, :])
                    nc.vector.tensor_mul(C[:, hp, q0:q0 + QBS], c_ab, r)
                return norm

            def make_outproj_tt(q0, tt):
                def outproj():
                    tg = q0 + tt * 128
                    o = op.tile([128, D], f32, tag="o")
                    ps0 = pp.tile([128, 512], f32, tag="pp", name="ps0")
                    ps1 = pp.tile([128, 512], f32, tag="pp", name="ps1")
                    for cc in range(2):
                        nc.tensor.matmul(ps0, C[:, cc, tg:tg + 128],
                                         wo_sb[:, cc, 0:512],
                                         start=(cc == 0), stop=(cc == 1))
                        nc.tensor.matmul(ps1, C[:, cc, tg:tg + 128],
                                         wo_sb[:, cc, 512:1024],
                                         start=(cc == 0), stop=(cc == 1))
                    nc.vector.tensor_copy(o[:, 0:512], ps0)
                    nc.vector.tensor_copy(o[:, 512:1024], ps1)
                    nc.sync.dma_start(out[tg:tg + 128, :], o)
                return outproj

            def make_qproj_late(tb, cc):
                def unit():
                    if cc == 0:
                        xq[tb] = x_load(qT, tb, nc.gpsimd, "xq")
                    make_kq_cc(wq_sb, bq_sb, xq[tb], QT, tb, cc)()
                return unit

            for qb in range(QB):
                q0 = qb * QBS
                for hp in range(2):
                    c_ab = cps.tile([128, QBS], f32, tag="cab", name="c_ab")
                    den = cps.tile([128, QBS], f32, tag="den", name="den")

                    def pv(kt, e, c_ab=c_ab, den=den, hp=hp):
                        # PV head pair col-packs into one PE window; the den
                        # pair (M=32 each, fixed homes) takes a second window
                        h0 = 128 * hp
                        nc.tensor.matmul(c_ab[0:64, :], V[:, kt, h0:h0 + 64],
                                         e[:, 0:QBS], start=(kt == 0),
                                         stop=(kt == KT_N - 1))
                        nc.tensor.matmul(c_ab[64:128, :],
                                         V[:, kt, h0 + 64:h0 + 128],
                                         e[:, QBS:2 * QBS], start=(kt == 0),
                                         stop=(kt == KT_N - 1))
                        nc.tensor.matmul(den[0:32, :], ones_sb,
                                         e[:, 0:QBS], start=(kt == 0),
                                         stop=(kt == KT_N - 1),
                                         tile_position=(0, 0))
                        nc.tensor.matmul(den[32:64, :], ones_sb,
                                         e[:, QBS:2 * QBS], start=(kt == 0),
                                         stop=(kt == KT_N - 1),
                                         tile_position=(0, 32))

                    # first chunk drains 2 units/kt so every V-proj unit is
                    # emitted before the PV flush that consumes it
                    drain = 2 if (qb == 0 and hp == 0) else 1
                    pending = []
                    for kt in range(KT_N):
                        k0 = kt * 128
                        s_ps = sp.tile([128, 2 * QBS], f32, tag="s")
                        nc.tensor.matmul(s_ps[:, 0:QBS],
                                         KT[0:64, hp, k0:k0 + 128],
                                         QT[0:64, hp, q0:q0 + QBS])
                        nc.tensor.matmul(s_ps[:, QBS:2 * QBS],
                                         KT[64:128, hp, k0:k0 + 128],
                                         QT[64:128, hp, q0:q0 + QBS])
                        e = ep.tile([128, 2 * QBS], bf16, tag="e")
                        nc.scalar.activation(e, s_ps, AF.Exp, scale=0.125)
                        pending.append((kt, e))
                        if kt >= 1:
                            for _ in range(drain):
                                if deferred:
                                    deferred.pop(0)()
                        if len(pending) > 4:
                            pv(*pending.pop(0))
                    for item in pending:
                        pv(*item)
                    deferred.insert(0, make_norm(hp, q0, c_ab, den))
                    if qb + 1 < QB:
                        deferred.insert(1, make_qproj_late(qb + 1, hp))
                for tt in range(4):
                    deferred.append(make_outproj_tt(q0, tt))
            for fn in deferred:
                fn()

    nc.compile()
    return nc


_CACHE = {}


def _get_nc():
    if "nc" not in _CACHE:
        _CACHE["nc"] = build()
    return _CACHE["nc"]


def make_in_maps(q, k, v, wq, bq, wk, bk, wv, bv, wo, bo):
    xT = {}
    for b in range(B):
        xT[("q", b)] = np.ascontiguousarray(np.asarray(q[b]).T).astype(BF)
        xT[("k", b)] = np.ascontiguousarray(np.asarray(k[b]).T).astype(BF)
        xT[("v", b)] = np.ascontiguousarray(np.asarray(v[b]).T).astype(BF)
    in_maps = []
    for core in range(N_CORES):
        b, j = divmod(core, N_CORES // B)
        sl = slice(CL * j, CL * (j + 1))
        in_maps.append({
            "qT": xT[("q", b)],
            "kT": xT[("k", b)],
            "vT": xT[("v", b)],
            "wqT": np.ascontiguousarray(np.asarray(wq)[sl, :].T).astype(BF),
            "wkT": np.ascontiguousarray(np.asarray(wk)[sl, :].T).astype(BF),
            "wvT": np.ascontiguousarray(np.asarray(wv)[sl, :].T).astype(BF),
            "woT": np.ascontiguousarray(np.asarray(wo)[:, sl].T).astype(BF),
            "bq": np.ascontiguousarray(bq[sl], dtype=np.float32),
            "bk": np.ascontiguousarray(bk[sl], dtype=np.float32),
        })
    return in_maps


def combine(results, bv, wo, bo):
    GP = N_CORES // B
    const = (np.asarray(bv, dtype=np.float64) @ np.asarray(wo, dtype=np.float64).T
             + np.asarray(bo, dtype=np.float64)).astype(np.float32)
    out = np.empty((B, S, D), dtype=np.float32)
    for b in range(B):
        acc = results[b * GP]["out"].astype(np.float32).copy()
        for j in range(1, GP):
            acc += results[b * GP + j]["out"]
        out[b] = acc + const[None, :]
    return out


def kernel(q, k, v, wq, bq, wk, bk, wv, bv, wo, bo):
    nc = _get_nc()
    in_maps = make_in_maps(q, k, v, wq, bq, wk, bk, wv, bv, wo, bo)
    res = bass_utils.run_bass_kernel_spmd(nc, in_maps, core_ids=list(range(N_CORES)))
    return combine(res.results, bv, wo, bo)
